# revision 25
# baseline (speedup 1.0000x reference)
"""CARAFE (content-aware reassembly of features) Trainium2 Bass kernel.

Full inputs in, full output out. Internally: pure data-parallel sharding
across 8 NeuronCores — core i handles batch b=i//2, H-half i%2 (32 input
rows -> 64 output rows), with a 2-row halo on the x shard.

Per-core pipeline (all on one NeuronCore, SPMD identical program),
software-pipelined in 4 blocks of 8 input rows (4 row-pairs) each:
  1. 1x1 conv (PE, bf16)  -> BN+ReLU (ACT) -> h bf16    (64, 34 x 66)
  2. per block: 3x3 conv (PE bf16, 9 taps PSUM-accum) -> exp (ACT) -> E
  3. per-s sums over k*k=25 (PE f32r blockdiag matmul), transpose
     exp+sums to pixel-major (PE), reciprocal (DVE)
  4. normalized kernels (DVE tensor_mul, bf16, dj-major s-innermost),
     then partition-shifted SBUF->SBUF DMA copies (per dj) so the diag
     builds read kernel weights at the shifted partition
  5. reassembly: per row-pair r, 5 PSUM-accumulated bf16 matmuls (one
     per di) with BANDED rhs: band[q, p*4+s] = sum_dj w[di,dj,s,p] at
     q = p+dj-2. Bands live in persistent pre-zeroed tiles; each of the
     5 dj sub-diagonals is written by one DVE copy_predicated (mask =
     I4S[dj], a static shifted identity with w-edge masks baked in;
     data = KNS[dj] broadcast). Zeros off the band survive reuse since
     the support is static. x host-pre-transposed to pixel-major bf16
  6. pixel-shuffle copy from PSUM (ACT) and DMA out.
Block b+1's PE front-work is emitted before block b's reassembly so the
engine queues overlap DVE diag-builds with PE conv/transpose work.
"""

import os
import sys
from contextlib import ExitStack

import numpy as np

sys.path.insert(0, "/opt/trn_rl_repo")

import concourse.bass as bass  # noqa: E402
import concourse.bacc as bacc  # noqa: E402
import concourse.tile as tile  # noqa: E402
from concourse import mybir  # noqa: E402

import ml_dtypes  # noqa: E402

F32 = mybir.dt.float32
F32R = mybir.dt.float32r
BF16 = mybir.dt.bfloat16
U8 = mybir.dt.uint8
NP_BF16 = ml_dtypes.bfloat16

# geometry (hardcoded for nn_CARAFEFast: x (4,128,64,64), w1 (64,128),
# w2 (100,64,3,3), S=2, K=5)
B, C, H, W = 4, 128, 64, 64
CM = 64          # c_mid
S, KUP = 2, 5    # upsample scale, reassembly kernel
NK = KUP * KUP   # 25
NS = S * S       # 4
NCH = NS * NK    # 100 kernel channels
NCORES = 8

RH = H // 2            # input rows of output region per core = 32
XR = RH + 4            # x-shard rows (2-halo each side) = 36
HR = RH + 2            # h rows (conv3x3 needs +-1) = 34
WP = W + 2             # W padded = 66
HCOLS = 4 + HR * WP + 4  # h flat cols (+4 pad head/tail for shifted conv APs)
NTE = XR // 2          # even row-pair tiles of x = 18
NTO = (XR - 2) // 2    # odd row-pair tiles = 17
NR = RH // 2           # output row-pair tiles = 16
KTW = NCH + NS         # 104: exp channels + per-s sums
NBLK = 4               # pipeline blocks
BR = NR // NBLK        # row-pairs per block = 4
BROWS = 2 * BR         # input rows per block = 8
BCOLS = BROWS * W      # E cols per block = 512
KNB = BR * NS * KUP    # 80: per-dj block of kernel weights (s innermost)

_CACHE: dict = {}


def _emit(ctx, tc):
    nc = tc.nc

    # ---- DRAM I/O ----
    xs_d = nc.dram_tensor("xs", [C, 8 + XR * W], BF16, kind="ExternalInput")
    xte_d = nc.dram_tensor("xte", [C, NTE * C], BF16, kind="ExternalInput")
    xto_d = nc.dram_tensor("xto", [C, NTO * C], BF16, kind="ExternalInput")
    i4_d = nc.dram_tensor("i4", [C, KUP * NS * C], BF16, kind="ExternalInput")
    w1t_d = nc.dram_tensor("w1t", [C, CM], BF16, kind="ExternalInput")
    w2l_d = nc.dram_tensor("w2l", [CM, 9 * NCH], BF16, kind="ExternalInput")
    bns_d = nc.dram_tensor("bns", [CM, 1], F32, kind="ExternalInput")
    bnb_d = nc.dram_tensor("bnb", [CM, 1], F32, kind="ExternalInput")
    be_d = nc.dram_tensor("be", [CM, 4], F32, kind="ExternalInput")
    bd_d = nc.dram_tensor("bd", [NCH, NS], F32R, kind="ExternalInput")
    i2_d = nc.dram_tensor("i2", [2, 4], BF16, kind="ExternalInput")
    i2r_d = nc.dram_tensor("i2r", [2, 4], F32R, kind="ExternalInput")
    idmf_d = nc.dram_tensor("idmf", [C, C], F32R, kind="ExternalInput")
    o_d = nc.dram_tensor("o", [C, 2 * RH * 2 * W], F32, kind="ExternalOutput")

    # ---- SBUF persistent tensors ----
    consts = ctx.enter_context(tc.tile_pool(name="consts", bufs=1))
    big = ctx.enter_context(tc.tile_pool(name="big", bufs=1))

    W1T = consts.tile([C, CM], BF16, tag="w1t")
    W2L = consts.tile([CM, 9 * NCH], BF16, tag="w2l")
    BNS = consts.tile([CM, 1], F32, tag="bns")
    BNB = consts.tile([CM, 1], F32, tag="bnb")
    BE = consts.tile([CM, 4], F32, tag="be")
    BD = consts.tile([NCH, NS], F32R, tag="bd")
    I2 = consts.tile([2, 4], BF16, tag="i2")
    I2R = consts.tile([2, 4], F32R, tag="i2r")
    IDMF = consts.tile([C, C], F32R, tag="idmf")
    # static shifted 1x4-block identities, one per dj, edge masks baked in:
    # I4S[dj][q, p*4+s] = 1 iff p == q-(dj-2) and the w-shift stays in-image
    I4S = consts.tile([C, KUP * NS * C], BF16, tag="i4s")
    DUM = consts.tile([C, 512], BF16, tag="dum")

    XS = big.tile([C, 8 + XR * W], BF16, tag="xs")
    HH = big.tile([CM, HCOLS], BF16, tag="hh")
    # host-pre-transposed x, pixel-major bf16: partition q = 64*par + w
    XTE = big.tile([C, NTE * C], BF16, tag="xte")
    XTO = big.tile([C, NTO * C], BF16, tag="xto")
    # per-block tensors (exact dep tracking across the pipeline)
    EB = [big.tile([NCH, BCOLS], F32R, tag=f"e{b}", name=f"e{b}")
          for b in range(NBLK)]
    DB = [big.tile([NS, BCOLS], F32R, tag=f"d{b}", name=f"d{b}")
          for b in range(NBLK)]
    KTB = [big.tile([C, BR * KTW], F32, tag=f"kt{b}", name=f"kt{b}")
           for b in range(NBLK)]
    RCB = [big.tile([C, BR * NS], F32, tag=f"rc{b}", name=f"rc{b}")
           for b in range(NBLK)]
    # normalized kernel weights bf16, dj-major blocks, s innermost:
    # col = dj*KNB + r*20 + di*4 + s ; KNS partition-shifted by sh=dj-2
    KNU = [big.tile([C, 4 * KNB], BF16, tag=f"knu{b}", name=f"knu{b}")
           for b in range(NBLK)]
    KNS = [big.tile([C, KUP * KNB], BF16, tag=f"kns{b}", name=f"kns{b}")
           for b in range(NBLK)]

    ost_pool = ctx.enter_context(tc.tile_pool(name="ost", bufs=3))
    dg_pool = ctx.enter_context(tc.tile_pool(name="dg", bufs=6))

    psA = ctx.enter_context(tc.tile_pool(name="psA", bufs=2, space="PSUM"))
    psB = ctx.enter_context(tc.tile_pool(name="psB", bufs=2, space="PSUM"))
    pso = ctx.enter_context(tc.tile_pool(name="pso", bufs=4, space="PSUM"))

    # ---- loads ----
    # small consts first (PE touches wait on them), then conv operands
    nc.sync.dma_start(I2R[:], i2r_d[:])
    nc.sync.dma_start(I2[:], i2_d[:])
    nc.sync.dma_start(W1T[:], w1t_d[:])
    nc.sync.dma_start(BNS[:], bns_d[:])
    nc.sync.dma_start(BNB[:], bnb_d[:])
    nc.sync.dma_start(BE[:], be_d[:])
    nc.sync.dma_start(XS[:], xs_d[:])
    nc.sync.dma_start(BD[:], bd_d[:])
    nc.sync.dma_start(IDMF[:], idmf_d[:])
    nc.sync.dma_start(W2L[:], w2l_d[:])
    nc.scalar.dma_start(XTE[:], xte_d[:])
    nc.scalar.dma_start(XTO[:], xto_d[:])
    nc.gpsimd.dma_start(I4S[:], i4_d[:])

    nc.vector.memset(DUM[:], 0.5)
    # zero h padding columns + KNS edge partitions (never DMA-written)
    nc.vector.memset(HH[:], 0.0)
    for b in range(NBLK):
        nc.vector.memset(KNS[b][:], 0.0)

    # HAM warm-up: the PE clock sits at 1.2 GHz until ~3.4us of sustained
    # activity; burn the DMA-load dead time with dummy matmuls so the real
    # work starts at 2.4 GHz.
    scrw = psA.tile([CM, 512], F32, tag="psA", name="scrw")
    for i in range(40):
        nc.tensor.matmul(scrw[0:CM, 0:512], DUM[:, 0:CM], DUM[:, 0:512],
                         start=(i == 0), stop=(i == 39))

    # PE "touch" matmuls: absorb each const's DMA sem on the PE clock one at
    # a time (walrus allows a single sync-wait per LDWEIGHTS).
    scr = psA.tile([CM, 512], F32, tag="psA", name="scr")
    for i, cst in enumerate((I2R, BD)):
        nc.tensor.matmul(scr[0:2, 4 * i : 4 * i + 4], cst[0:2, 0:2],
                         I2R[0:2, 0:4], start=True, stop=True)
    nc.tensor.matmul(scr[0:2, 8:12], IDMF[0:2, 0:2], IDMF[0:2, 0:4],
                     start=True, stop=True)
    for i, cst in enumerate((I2, W1T, W2L)):
        nc.tensor.matmul(scr[0:2, 12 + 4 * i : 16 + 4 * i], cst[0:2, 0:2],
                         I2[0:2, 0:4], start=True, stop=True)

    def touch_xt():
        scr2 = psA.tile([CM, 512], F32, tag="psA", name="scr2")
        for i, cst in enumerate((XTE, XTO)):
            nc.tensor.matmul(scr2[0:2, 4 * i : 4 * i + 4], cst[0:2, 0:2],
                             I2[0:2, 0:4], start=True, stop=True)

    relu = mybir.ActivationFunctionType.Relu
    expf = mybir.ActivationFunctionType.Exp

    # ---- 1x1 conv + BN + ReLU -> HH bf16 (zero w-padding columns) ----
    hh3 = HH[:, 4 : 4 + HR * WP].rearrange("p (g w) -> p g w", w=WP)
    # pixels: x rows 1..34 (row 0 = r0-2 halo), i.e. XS cols [64, 64+34*64)
    a = 0
    while a < HR * W:
        n = min(512, HR * W - a)
        ps = psA.tile([CM, 512], F32, tag="psA", name="ps")
        nc.tensor.matmul(ps[:, 0:n], W1T[:], XS[:, 4 + W + a : 4 + W + a + n],
                         start=True, stop=True)
        g0, ng = a // W, n // W
        nc.scalar.activation(
            hh3[:, g0 : g0 + ng, 1 : 1 + W],
            ps[:, 0:n].rearrange("p (g w) -> p g w", w=W),
            relu, bias=BNB[:], scale=BNS[:],
        )
        a += n

    # boundary h rows (image edge padding): rows 0 and HR-1 recomputed with
    # per-core scale/bias (zeroed when the row is outside the image)
    for row, sc_i, bi_i in ((0, 0, 1), (HR - 1, 2, 3)):
        pb = psA.tile([CM, 512], F32, tag="psA", name="pb")
        nc.tensor.matmul(pb[:, 0:W], W1T[:],
                         XS[:, 4 + W + row * W : 4 + W + (row + 1) * W],
                         start=True, stop=True)
        nc.scalar.activation(hh3[:, row : row + 1, 1 : 1 + W],
                             pb[:, 0:W].rearrange("p (g w) -> p g w", w=W),
                             relu, bias=BE[:, bi_i : bi_i + 1],
                             scale=BE[:, sc_i : sc_i + 1])

    def front(b):
        """conv3x3+exp, sums, KT transpose, reciprocal, KN, KNS for block b."""
        E, D, KT, RC = EB[b], DB[b], KTB[b], RCB[b]
        # 3x3 conv (2 chunks of 4 rows, 9 taps PSUM-accum) + exp
        e3 = E[:].rearrange("p (g w) -> p g w", w=W)
        for ci in range(BROWS // 4):
            g0 = b * BROWS + ci * 4           # global row
            a, n = g0 * WP, 4 * WP
            pk = psB.tile([NCH, 4 * WP], F32, tag="psB", name="psk")
            for t in range(9):
                di, dj = t // 3, t % 3
                off = 4 + di * WP + dj - 1
                nc.tensor.matmul(pk[:, 0:n], W2L[:, t * NCH : (t + 1) * NCH],
                                 HH[:, off + a : off + a + n],
                                 start=(t == 0), stop=(t == 8))
            nc.scalar.activation(
                e3[:, ci * 4 : ci * 4 + 4, :],
                pk[0:NCH, 0:n].rearrange("p (g w) -> p g w", w=WP)[:, :, 1 : 1 + W],
                expf)
        # per-s sums over the 25-tap groups (f32r for 2cyc/col)
        pd = psA.tile([CM, 512], F32, tag="psA", name="pd")
        nc.tensor.matmul(pd[0:NS, 0:BCOLS], BD[:], E[:],
                         start=True, stop=True)
        nc.scalar.copy(D[:], pd[0:NS, 0:BCOLS])
        # transpose exp+sums to pixel-major KT
        pt = psB.tile([C, 512], F32R, tag="psB", name="pt")
        for rl in range(BR):
            c0 = rl * KTW
            nc.tensor.transpose(pt[:, c0 : c0 + NCH],
                                E[:, 2 * rl * W : 2 * (rl + 1) * W],
                                IDMF[0:NCH, 0:NCH])
            nc.tensor.transpose(pt[:, c0 + NCH : c0 + KTW],
                                D[:, 2 * rl * W : 2 * (rl + 1) * W],
                                IDMF[0:NS, 0:NS])
        nc.scalar.copy(KT[:], pt[:, 0 : BR * KTW].bitcast(F32))
        # reciprocal of sums
        kt3 = KT[:].rearrange("p (r c) -> p r c", c=KTW)
        rc3 = RC[:].rearrange("p (r s) -> p r s", s=NS)
        nc.vector.reciprocal(rc3[:], kt3[:, :, NCH:KTW])
        # normalized kernel weights: dj-major blocks, s innermost
        # dst[:, blk + r*20 + di*4 + s] = KT[p, r*104+s*25+di*5+dj] * RC[p, r*4+s]
        for dj in range(KUP):
            if dj == 2:
                dst_t, off = KNS[b], 2 * KNB
            else:
                dst_t, off = KNU[b], (dj if dj < 2 else dj - 1) * KNB
            dst = bass.AP(dst_t.tensor, dst_t.offset + off,
                          [list(dst_t.ap[0]), [20, BR], [4, KUP], [1, NS]])
            kt_src = bass.AP(KT.tensor, KT.offset + dj,
                             [list(KT.ap[0]), [KTW, BR], [5, KUP], [25, NS]])
            rc_src = bass.AP(RC.tensor, RC.offset,
                             [list(RC.ap[0]), [NS, BR], [0, KUP], [1, NS]])
            nc.vector.tensor_mul(dst, kt_src, rc_src)
        # partition-shifted SBUF->SBUF copies: KNS[dj][q,:] = KNU-blk[q-sh,:]
        for dj in (0, 1, 3, 4):
            sh = dj - 2
            m = dj if dj < 2 else dj - 1
            if sh < 0:
                nc.sync.dma_start(
                    KNS[b][0 : C + sh, dj * KNB : (dj + 1) * KNB],
                    KNU[b][-sh : C, m * KNB : (m + 1) * KNB])
            else:
                nc.sync.dma_start(
                    KNS[b][sh:C, dj * KNB : (dj + 1) * KNB],
                    KNU[b][0 : C - sh, m * KNB : (m + 1) * KNB])

    def tap_src(r, di):
        if di % 2 == 0:
            t = r + di // 2
            return XTE[:, t * C : (t + 1) * C]
        u = r + (di - 1) // 2
        return XTO[:, u * C : (u + 1) * C]

    def reassembly(b):
        """25 PSUM-accumulated bf16 diag-matmuls per row-pair, 4 row-pairs.

        po[c, p*4+s] = sum_taps XT_tap[q, c] * w[s, p],  q = p + (dj-2)
        DG[q, (rl, p*4+s)] built in ONE tensor_mul per tap (covers BR=4
        row-pairs): DG = I4S[dj] (static) * KNS[dj] (bcast over p).
        """
        r0 = b * BR
        po = [pso.tile([C, NS * C], F32, tag="pso", name=f"po{rr}")
              for rr in range(BR)]
        for di in range(KUP):
            srcs = [tap_src(r0 + rr, di) for rr in range(BR)]
            for dj in range(KUP):
                k_idx = di * KUP + dj
                dg = dg_pool.tile([C, BR * NS * C], BF16, tag="dg", name="dg")
                dst = bass.AP(dg.tensor, dg.offset,
                              [list(dg.ap[0]), [512, BR], [4, C], [1, NS]])
                in1 = bass.AP(I4S.tensor, I4S.offset + dj * NS * C,
                              [list(I4S.ap[0]), [0, BR], [4, C], [1, NS]])
                in2 = bass.AP(KNS[b].tensor,
                              KNS[b].offset + dj * KNB + di * NS,
                              [list(KNS[b].ap[0]), [20, BR], [0, C], [1, NS]])
                nc.vector.tensor_mul(dst, in1, in2)
                for rr in range(BR):
                    nc.tensor.matmul(po[rr][:], srcs[rr],
                                     dg[:, rr * 512 : (rr + 1) * 512],
                                     start=(k_idx == 0), stop=(k_idx == NK - 1))
        # pixel shuffle + store (ACT; DVE is the busy engine here)
        # src col: (par*64+w)*4 + 2*si+sj ; dst col: (2*par+si)*128 + 2*w + sj
        for rr in range(BR):
            r = r0 + rr
            ost = ost_pool.tile([C, NS * C], F32, tag="ost", name="ost")
            src4 = po[rr][:].rearrange("p (par w si sj) -> p par si sj w",
                                       par=2, w=W, si=2, sj=2)
            dst4 = ost[:].rearrange("p (par si w sj) -> p par si sj w",
                                    par=2, si=2, sj=2)
            nc.scalar.copy(dst4[:], src4[:])
            nc.sync.dma_start(o_d[:, r * 512 : (r + 1) * 512], ost[:])

    # software pipeline: front(b+1) emitted before reassembly(b) so PE's
    # queue interleaves next block's conv work with this block's matmuls
    # serial emission: PE queue = all front work, then one dense ~100us
    # matmul stream (keeps the HAM clock-gate warm); DVE still overlaps via
    # per-block tiles and the dg ring
    for b in range(NBLK):
        front(b)
    touch_xt()
    for b in range(NBLK):
        reassembly(b)


def _build():
    if "nc" in _CACHE:
        return _CACHE["nc"]
    nc = bacc.Bacc("TRN2", target_bir_lowering=False, debug=False)
    with tile.TileContext(nc) as tc:
        with ExitStack() as ctx:
            _emit(ctx, tc)
    nc.compile()
    _CACHE["nc"] = nc
    return nc


def _host_prep(x, w1, w2, bn_gamma, bn_beta, bn_mean, bn_var):
    x = np.asarray(x, np.float32)
    w1 = np.asarray(w1, np.float32)
    w2 = np.asarray(w2, np.float32)
    inv = np.asarray(bn_gamma, np.float32) / np.sqrt(np.asarray(bn_var, np.float32) + 1e-5)
    bias = np.asarray(bn_beta, np.float32) - np.asarray(bn_mean, np.float32) * inv

    w1t = np.ascontiguousarray(w1.T).astype(NP_BF16)             # (128, 64)
    w2l = np.ascontiguousarray(
        w2.transpose(1, 2, 3, 0).reshape(CM, 9 * NCH)).astype(NP_BF16)
    bd = np.zeros((NCH, NS), np.float32)
    for s in range(NS):
        bd[s * NK : (s + 1) * NK, s] = 1.0
    i2 = np.zeros((2, 4), np.float32)
    i2[0, 0] = i2[1, 1] = 1.0
    idmf = np.eye(C, dtype=np.float32)

    # I4S[dj][q, p*4+s] = 1 iff p == q-sh, p in [0,128), same 64-block
    i4 = np.zeros((C, KUP * NS * C), np.float32)
    for dj in range(KUP):
        sh = dj - 2
        for q in range(C):
            p = q - sh
            if 0 <= p < C and p // 64 == q // 64:
                for s in range(NS):
                    i4[q, dj * NS * C + p * NS + s] = 1.0
    i4 = i4.astype(NP_BF16)

    xp = np.pad(x, ((0, 0), (0, 0), (2, 2), (0, 0)))             # H-halo zeros
    in_maps = []
    for core in range(NCORES):
        b, half = core // 2, core % 2
        r0 = half * RH
        xs = np.zeros((C, 8 + XR * W), np.float32)
        xs[:, 4 : 4 + XR * W] = xp[b, :, r0 : r0 + XR, :].reshape(C, XR * W)
        # pixel-major bf16 transposes of the shard (halo included):
        # XTE[64*par+w, t*128+c] = xp[b, c, r0+2t+par, w]
        xsh = xp[b, :, r0 : r0 + XR, :].transpose(1, 2, 0)       # (36, 64, 128)
        xte = np.ascontiguousarray(
            xsh[0 : 2 * NTE].reshape(NTE, C, C).transpose(1, 0, 2)
            .reshape(C, NTE * C)).astype(NP_BF16)
        xto = np.ascontiguousarray(
            xsh[1 : 1 + 2 * NTO].reshape(NTO, C, C).transpose(1, 0, 2)
            .reshape(C, NTO * C)).astype(NP_BF16)
        be = np.zeros((CM, 4), np.float32)
        if half == 0:
            be[:, 0] = 0.0            # h row 0 = image row -1 -> zero
            be[:, 1] = 0.0
            be[:, 2] = inv
            be[:, 3] = bias
        else:
            be[:, 0] = inv
            be[:, 1] = bias
            be[:, 2] = 0.0            # h row HR-1 = image row 64 -> zero
            be[:, 3] = 0.0
        in_maps.append({
            "xs": xs.astype(NP_BF16), "xte": xte, "xto": xto, "i4": i4,
            "w1t": w1t, "w2l": w2l,
            "bns": inv.reshape(CM, 1).astype(np.float32),
            "bnb": bias.reshape(CM, 1).astype(np.float32),
            "be": be, "bd": bd,
            "i2": i2.astype(NP_BF16), "i2r": i2, "idmf": idmf,
        })
    return in_maps


def _assemble(results):
    out = np.zeros((B, C, 2 * H, 2 * W), np.float32)
    for core in range(NCORES):
        b, half = core // 2, core % 2
        o = results[core]["o"].reshape(C, 2 * RH, 2 * W)
        out[b, :, half * 2 * RH : (half + 1) * 2 * RH, :] = o
    return out


def kernel(x, w1, w2, bn_gamma, bn_beta, bn_mean, bn_var):
    nc = _build()
    in_maps = _host_prep(x, w1, w2, bn_gamma, bn_beta, bn_mean, bn_var)

    if os.environ.get("CARAFE_BACKEND", "hw") == "sim":
        from concourse.bass_interp import CoreSim
        results = []
        for core in range(NCORES):
            sim = CoreSim(nc)
            for name, arr in in_maps[core].items():
                sim.tensor(name)[:] = arr
            sim.simulate()
            results.append({"o": np.array(sim.mem_tensor("o"))})
    else:
        from concourse.bass_utils import run_bass_kernel_spmd
        res = run_bass_kernel_spmd(nc, in_maps, core_ids=list(range(NCORES)))
        results = res.results
    return _assemble(results)


# revision 26
# speedup vs baseline: 1.0364x; 1.0364x over previous
"""CARAFE (content-aware reassembly of features) Trainium2 Bass kernel.

Full inputs in, full output out. Internally: pure data-parallel sharding
across 8 NeuronCores — core i handles batch b=i//2, H-half i%2 (32 input
rows -> 64 output rows), with a 2-row halo on the x shard.

Per-core pipeline (all on one NeuronCore, SPMD identical program),
software-pipelined in 4 blocks of 8 input rows (4 row-pairs) each:
  1. 1x1 conv (PE, bf16)  -> BN+ReLU (ACT) -> h bf16    (64, 34 x 66)
  2. per block: 3x3 conv (PE bf16, 9 taps PSUM-accum) -> exp (ACT) -> E
  3. per-s sums over k*k=25 (PE f32r blockdiag matmul), transpose
     exp+sums to pixel-major (PE), reciprocal (DVE)
  4. normalized kernels (DVE tensor_mul, bf16, dj-major s-innermost),
     then partition-shifted SBUF->SBUF DMA copies (per dj) so the diag
     builds read kernel weights at the shifted partition
  5. reassembly: per row-pair r, 5 PSUM-accumulated bf16 matmuls (one
     per di) with BANDED rhs: band[q, p*4+s] = sum_dj w[di,dj,s,p] at
     q = p+dj-2. Bands live in persistent pre-zeroed tiles; each of the
     5 dj sub-diagonals is written by one DVE copy_predicated (mask =
     I4S[dj], a static shifted identity with w-edge masks baked in;
     data = KNS[dj] broadcast). Zeros off the band survive reuse since
     the support is static. x host-pre-transposed to pixel-major bf16
  6. pixel-shuffle copy from PSUM (ACT) and DMA out.
Block b+1's PE front-work is emitted before block b's reassembly so the
engine queues overlap DVE diag-builds with PE conv/transpose work.
"""

import os
import sys
from contextlib import ExitStack

import numpy as np

sys.path.insert(0, "/opt/trn_rl_repo")

import concourse.bass as bass  # noqa: E402
import concourse.bacc as bacc  # noqa: E402
import concourse.tile as tile  # noqa: E402
from concourse import mybir  # noqa: E402

import ml_dtypes  # noqa: E402

F32 = mybir.dt.float32
F32R = mybir.dt.float32r
BF16 = mybir.dt.bfloat16
U8 = mybir.dt.uint8
NP_BF16 = ml_dtypes.bfloat16

# geometry (hardcoded for nn_CARAFEFast: x (4,128,64,64), w1 (64,128),
# w2 (100,64,3,3), S=2, K=5)
B, C, H, W = 4, 128, 64, 64
CM = 64          # c_mid
S, KUP = 2, 5    # upsample scale, reassembly kernel
NK = KUP * KUP   # 25
NS = S * S       # 4
NCH = NS * NK    # 100 kernel channels
NCORES = 8

RH = H // 2            # input rows of output region per core = 32
XR = RH + 4            # x-shard rows (2-halo each side) = 36
HR = RH + 2            # h rows (conv3x3 needs +-1) = 34
WP = W + 2             # W padded = 66
HCOLS = 4 + HR * WP + 4  # h flat cols (+4 pad head/tail for shifted conv APs)
NTE = XR // 2          # even row-pair tiles of x = 18
NTO = (XR - 2) // 2    # odd row-pair tiles = 17
NR = RH // 2           # output row-pair tiles = 16
KTW = NCH + NS         # 104: exp channels + per-s sums
NBLK = 4               # pipeline blocks
BR = NR // NBLK        # row-pairs per block = 4
BROWS = 2 * BR         # input rows per block = 8
BCOLS = BROWS * W      # E cols per block = 512
KNB = BR * NS * KUP    # 80: per-dj block of kernel weights (s innermost)

_CACHE: dict = {}


def _emit(ctx, tc):
    nc = tc.nc

    # ---- DRAM I/O ----
    xs_d = nc.dram_tensor("xs", [C, 8 + XR * W], BF16, kind="ExternalInput")
    xte_d = nc.dram_tensor("xte", [C, NTE * C], BF16, kind="ExternalInput")
    xto_d = nc.dram_tensor("xto", [C, NTO * C], BF16, kind="ExternalInput")
    i4_d = nc.dram_tensor("i4", [C, KUP * NS * C], BF16, kind="ExternalInput")
    w1t_d = nc.dram_tensor("w1t", [C, CM], BF16, kind="ExternalInput")
    w2l_d = nc.dram_tensor("w2l", [CM, 9 * NCH], BF16, kind="ExternalInput")
    bns_d = nc.dram_tensor("bns", [CM, 1], F32, kind="ExternalInput")
    bnb_d = nc.dram_tensor("bnb", [CM, 1], F32, kind="ExternalInput")
    be_d = nc.dram_tensor("be", [CM, 4], F32, kind="ExternalInput")
    bd_d = nc.dram_tensor("bd", [NCH, NS], F32R, kind="ExternalInput")
    i2_d = nc.dram_tensor("i2", [2, 4], BF16, kind="ExternalInput")
    i2r_d = nc.dram_tensor("i2r", [2, 4], F32R, kind="ExternalInput")
    idmf_d = nc.dram_tensor("idmf", [C, C], F32R, kind="ExternalInput")
    o_d = nc.dram_tensor("o", [C, 2 * RH * 2 * W], F32, kind="ExternalOutput")

    # ---- SBUF persistent tensors ----
    consts = ctx.enter_context(tc.tile_pool(name="consts", bufs=1))
    big = ctx.enter_context(tc.tile_pool(name="big", bufs=1))

    W1T = consts.tile([C, CM], BF16, tag="w1t")
    W2L = consts.tile([CM, 9 * NCH], BF16, tag="w2l")
    BNS = consts.tile([CM, 1], F32, tag="bns")
    BNB = consts.tile([CM, 1], F32, tag="bnb")
    BE = consts.tile([CM, 4], F32, tag="be")
    BD = consts.tile([NCH, NS], F32R, tag="bd")
    I2 = consts.tile([2, 4], BF16, tag="i2")
    I2R = consts.tile([2, 4], F32R, tag="i2r")
    IDMF = consts.tile([C, C], F32R, tag="idmf")
    # static shifted 1x4-block identities, one per dj, edge masks baked in:
    # I4S[dj][q, p*4+s] = 1 iff p == q-(dj-2) and the w-shift stays in-image
    I4S = consts.tile([C, KUP * NS * C], BF16, tag="i4s")
    DUM = consts.tile([C, 512], BF16, tag="dum")

    XS = big.tile([C, 8 + XR * W], BF16, tag="xs")
    HH = big.tile([CM, HCOLS], BF16, tag="hh")
    # host-pre-transposed x, pixel-major bf16: partition q = 64*par + w
    XTE = big.tile([C, NTE * C], BF16, tag="xte")
    XTO = big.tile([C, NTO * C], BF16, tag="xto")
    # per-block tensors (exact dep tracking across the pipeline)
    EB = [big.tile([NCH, BCOLS], F32R, tag=f"e{b}", name=f"e{b}")
          for b in range(NBLK)]
    DB = [big.tile([NS, BCOLS], F32R, tag=f"d{b}", name=f"d{b}")
          for b in range(NBLK)]
    KTB = [big.tile([C, BR * KTW], F32, tag=f"kt{b}", name=f"kt{b}")
           for b in range(NBLK)]
    RCB = [big.tile([C, BR * NS], F32, tag=f"rc{b}", name=f"rc{b}")
           for b in range(NBLK)]
    # normalized kernel weights bf16, dj-major blocks, s innermost:
    # col = dj*KNB + r*20 + di*4 + s ; KNS partition-shifted by sh=dj-2
    KNU = [big.tile([C, 4 * KNB], BF16, tag=f"knu{b}", name=f"knu{b}")
           for b in range(NBLK)]
    KNS = [big.tile([C, KUP * KNB], BF16, tag=f"kns{b}", name=f"kns{b}")
           for b in range(NBLK)]

    ost_pool = ctx.enter_context(tc.tile_pool(name="ost", bufs=3))
    dg_pool = ctx.enter_context(tc.tile_pool(name="dg", bufs=6))

    psA = ctx.enter_context(tc.tile_pool(name="psA", bufs=2, space="PSUM"))
    psB = ctx.enter_context(tc.tile_pool(name="psB", bufs=2, space="PSUM"))
    pso = ctx.enter_context(tc.tile_pool(name="pso", bufs=4, space="PSUM"))

    # ---- loads ----
    # small consts first (PE touches wait on them), then conv operands
    nc.sync.dma_start(I2R[:], i2r_d[:])
    nc.sync.dma_start(I2[:], i2_d[:])
    nc.sync.dma_start(W1T[:], w1t_d[:])
    nc.sync.dma_start(BNS[:], bns_d[:])
    nc.sync.dma_start(BNB[:], bnb_d[:])
    nc.sync.dma_start(BE[:], be_d[:])
    nc.sync.dma_start(XS[:], xs_d[:])
    nc.sync.dma_start(BD[:], bd_d[:])
    nc.sync.dma_start(IDMF[:], idmf_d[:])
    nc.sync.dma_start(W2L[:], w2l_d[:])
    nc.scalar.dma_start(XTE[:], xte_d[:])
    nc.scalar.dma_start(XTO[:], xto_d[:])
    nc.gpsimd.dma_start(I4S[:], i4_d[:])

    nc.vector.memset(DUM[:], 0.5)
    # zero h padding columns + KNS edge partitions (never DMA-written)
    nc.vector.memset(HH[:], 0.0)
    for b in range(NBLK):
        nc.vector.memset(KNS[b][:], 0.0)

    # HAM warm-up: the PE clock sits at 1.2 GHz until ~3.4us of sustained
    # activity; burn the DMA-load dead time with dummy matmuls so the real
    # work starts at 2.4 GHz.
    scrw = psA.tile([CM, 512], F32, tag="psA", name="scrw")
    for i in range(40):
        nc.tensor.matmul(scrw[0:CM, 0:512], DUM[:, 0:CM], DUM[:, 0:512],
                         start=(i == 0), stop=(i == 39))

    # PE "touch" matmuls: absorb each const's DMA sem on the PE clock one at
    # a time (walrus allows a single sync-wait per LDWEIGHTS).
    scr = psA.tile([CM, 512], F32, tag="psA", name="scr")
    for i, cst in enumerate((I2R, BD)):
        nc.tensor.matmul(scr[0:2, 4 * i : 4 * i + 4], cst[0:2, 0:2],
                         I2R[0:2, 0:4], start=True, stop=True)
    nc.tensor.matmul(scr[0:2, 8:12], IDMF[0:2, 0:2], IDMF[0:2, 0:4],
                     start=True, stop=True)
    for i, cst in enumerate((I2, W1T, W2L)):
        nc.tensor.matmul(scr[0:2, 12 + 4 * i : 16 + 4 * i], cst[0:2, 0:2],
                         I2[0:2, 0:4], start=True, stop=True)

    def touch_xt():
        scr2 = psA.tile([CM, 512], F32, tag="psA", name="scr2")
        for i, cst in enumerate((XTE, XTO)):
            nc.tensor.matmul(scr2[0:2, 4 * i : 4 * i + 4], cst[0:2, 0:2],
                             I2[0:2, 0:4], start=True, stop=True)

    relu = mybir.ActivationFunctionType.Relu
    expf = mybir.ActivationFunctionType.Exp

    # ---- 1x1 conv + BN + ReLU -> HH bf16 (zero w-padding columns) ----
    hh3 = HH[:, 4 : 4 + HR * WP].rearrange("p (g w) -> p g w", w=WP)
    # pixels: x rows 1..34 (row 0 = r0-2 halo), i.e. XS cols [64, 64+34*64)
    a = 0
    while a < HR * W:
        n = min(512, HR * W - a)
        ps = psA.tile([CM, 512], F32, tag="psA", name="ps")
        nc.tensor.matmul(ps[:, 0:n], W1T[:], XS[:, 4 + W + a : 4 + W + a + n],
                         start=True, stop=True)
        g0, ng = a // W, n // W
        nc.scalar.activation(
            hh3[:, g0 : g0 + ng, 1 : 1 + W],
            ps[:, 0:n].rearrange("p (g w) -> p g w", w=W),
            relu, bias=BNB[:], scale=BNS[:],
        )
        a += n

    # boundary h rows (image edge padding): rows 0 and HR-1 recomputed with
    # per-core scale/bias (zeroed when the row is outside the image)
    for row, sc_i, bi_i in ((0, 0, 1), (HR - 1, 2, 3)):
        pb = psA.tile([CM, 512], F32, tag="psA", name="pb")
        nc.tensor.matmul(pb[:, 0:W], W1T[:],
                         XS[:, 4 + W + row * W : 4 + W + (row + 1) * W],
                         start=True, stop=True)
        nc.scalar.activation(hh3[:, row : row + 1, 1 : 1 + W],
                             pb[:, 0:W].rearrange("p (g w) -> p g w", w=W),
                             relu, bias=BE[:, bi_i : bi_i + 1],
                             scale=BE[:, sc_i : sc_i + 1])

    def front(b):
        """conv3x3+exp, sums, KT transpose, reciprocal, KN, KNS for block b."""
        E, D, KT, RC = EB[b], DB[b], KTB[b], RCB[b]
        # 3x3 conv (2 chunks of 4 rows, 9 taps PSUM-accum) + exp
        e3 = E[:].rearrange("p (g w) -> p g w", w=W)
        for ci in range(BROWS // 4):
            g0 = b * BROWS + ci * 4           # global row
            a, n = g0 * WP, 4 * WP
            pk = psB.tile([NCH, 4 * WP], F32, tag="psB", name="psk")
            for t in range(9):
                di, dj = t // 3, t % 3
                off = 4 + di * WP + dj - 1
                nc.tensor.matmul(pk[:, 0:n], W2L[:, t * NCH : (t + 1) * NCH],
                                 HH[:, off + a : off + a + n],
                                 start=(t == 0), stop=(t == 8))
            nc.scalar.activation(
                e3[:, ci * 4 : ci * 4 + 4, :],
                pk[0:NCH, 0:n].rearrange("p (g w) -> p g w", w=WP)[:, :, 1 : 1 + W],
                expf)
        # per-s sums over the 25-tap groups (f32r for 2cyc/col)
        pd = psA.tile([CM, 512], F32, tag="psA", name="pd")
        nc.tensor.matmul(pd[0:NS, 0:BCOLS], BD[:], E[:],
                         start=True, stop=True)
        nc.scalar.copy(D[:], pd[0:NS, 0:BCOLS])
        # transpose exp+sums to pixel-major KT
        pt = psB.tile([C, 512], F32R, tag="psB", name="pt")
        for rl in range(BR):
            c0 = rl * KTW
            nc.tensor.transpose(pt[:, c0 : c0 + NCH],
                                E[:, 2 * rl * W : 2 * (rl + 1) * W],
                                IDMF[0:NCH, 0:NCH])
            nc.tensor.transpose(pt[:, c0 + NCH : c0 + KTW],
                                D[:, 2 * rl * W : 2 * (rl + 1) * W],
                                IDMF[0:NS, 0:NS])
        nc.scalar.copy(KT[:], pt[:, 0 : BR * KTW].bitcast(F32))
        # reciprocal of sums
        kt3 = KT[:].rearrange("p (r c) -> p r c", c=KTW)
        rc3 = RC[:].rearrange("p (r s) -> p r s", s=NS)
        nc.vector.reciprocal(rc3[:], kt3[:, :, NCH:KTW])
        # normalized kernel weights: dj-major blocks, s innermost
        # dst[:, blk + r*20 + di*4 + s] = KT[p, r*104+s*25+di*5+dj] * RC[p, r*4+s]
        for dj in range(KUP):
            if dj == 2:
                dst_t, off = KNS[b], 2 * KNB
            else:
                dst_t, off = KNU[b], (dj if dj < 2 else dj - 1) * KNB
            dst = bass.AP(dst_t.tensor, dst_t.offset + off,
                          [list(dst_t.ap[0]), [20, BR], [4, KUP], [1, NS]])
            kt_src = bass.AP(KT.tensor, KT.offset + dj,
                             [list(KT.ap[0]), [KTW, BR], [5, KUP], [25, NS]])
            rc_src = bass.AP(RC.tensor, RC.offset,
                             [list(RC.ap[0]), [NS, BR], [0, KUP], [1, NS]])
            nc.vector.tensor_mul(dst, kt_src, rc_src)
        # partition-shifted SBUF->SBUF copies: KNS[dj][q,:] = KNU-blk[q-sh,:]
        for dj in (0, 1, 3, 4):
            sh = dj - 2
            m = dj if dj < 2 else dj - 1
            if sh < 0:
                nc.sync.dma_start(
                    KNS[b][0 : C + sh, dj * KNB : (dj + 1) * KNB],
                    KNU[b][-sh : C, m * KNB : (m + 1) * KNB])
            else:
                nc.sync.dma_start(
                    KNS[b][sh:C, dj * KNB : (dj + 1) * KNB],
                    KNU[b][0 : C - sh, m * KNB : (m + 1) * KNB])

    def tap_src(r, di):
        if di % 2 == 0:
            t = r + di // 2
            return XTE[:, t * C : (t + 1) * C]
        u = r + (di - 1) // 2
        return XTO[:, u * C : (u + 1) * C]

    def reassembly(b):
        """25 PSUM-accumulated bf16 diag-matmuls per row-pair, 4 row-pairs.

        po[c, p*4+s] = sum_taps XT_tap[q, c] * w[s, p],  q = p + (dj-2)
        DG[q, (rl, p*4+s)] built in ONE tensor_mul per tap (covers BR=4
        row-pairs): DG = I4S[dj] (static) * KNS[dj] (bcast over p).
        """
        r0 = b * BR
        po = [pso.tile([C, NS * C], F32, tag="pso", name=f"po{rr}")
              for rr in range(BR)]
        for di in range(KUP):
            srcs = [tap_src(r0 + rr, di) for rr in range(BR)]
            for dj in range(KUP):
                k_idx = di * KUP + dj
                dg = dg_pool.tile([C, BR * NS * C], BF16, tag="dg", name="dg")
                dst = bass.AP(dg.tensor, dg.offset,
                              [list(dg.ap[0]), [512, BR], [4, C], [1, NS]])
                in1 = bass.AP(I4S.tensor, I4S.offset + dj * NS * C,
                              [list(I4S.ap[0]), [0, BR], [4, C], [1, NS]])
                in2 = bass.AP(KNS[b].tensor,
                              KNS[b].offset + dj * KNB + di * NS,
                              [list(KNS[b].ap[0]), [20, BR], [0, C], [1, NS]])
                nc.vector.tensor_mul(dst, in1, in2)
                for rr in range(BR):
                    nc.tensor.matmul(po[rr][:], srcs[rr],
                                     dg[:, rr * 512 : (rr + 1) * 512],
                                     start=(k_idx == 0), stop=(k_idx == NK - 1))
        # pixel shuffle + store (ACT; DVE is the busy engine here)
        # src col: (par*64+w)*4 + 2*si+sj ; dst col: (2*par+si)*128 + 2*w + sj
        for rr in range(BR):
            r = r0 + rr
            ost = ost_pool.tile([C, NS * C], F32, tag="ost", name="ost")
            src4 = po[rr][:].rearrange("p (par w si sj) -> p par si sj w",
                                       par=2, w=W, si=2, sj=2)
            dst4 = ost[:].rearrange("p (par si w sj) -> p par si sj w",
                                    par=2, si=2, sj=2)
            nc.scalar.copy(dst4[:], src4[:])
            nc.sync.dma_start(o_d[:, r * 512 : (r + 1) * 512], ost[:])

    # software pipeline: front(b+1) emitted before reassembly(b) so PE's
    # queue interleaves next block's conv work with this block's matmuls
    front(0)
    front(1)
    touch_xt()
    for b in range(NBLK):
        if b + 2 < NBLK:
            front(b + 2)
        reassembly(b)


def _build():
    if "nc" in _CACHE:
        return _CACHE["nc"]
    nc = bacc.Bacc("TRN2", target_bir_lowering=False, debug=False)
    with tile.TileContext(nc) as tc:
        with ExitStack() as ctx:
            _emit(ctx, tc)
    nc.compile()
    _CACHE["nc"] = nc
    return nc


def _host_prep(x, w1, w2, bn_gamma, bn_beta, bn_mean, bn_var):
    x = np.asarray(x, np.float32)
    w1 = np.asarray(w1, np.float32)
    w2 = np.asarray(w2, np.float32)
    inv = np.asarray(bn_gamma, np.float32) / np.sqrt(np.asarray(bn_var, np.float32) + 1e-5)
    bias = np.asarray(bn_beta, np.float32) - np.asarray(bn_mean, np.float32) * inv

    w1t = np.ascontiguousarray(w1.T).astype(NP_BF16)             # (128, 64)
    w2l = np.ascontiguousarray(
        w2.transpose(1, 2, 3, 0).reshape(CM, 9 * NCH)).astype(NP_BF16)
    bd = np.zeros((NCH, NS), np.float32)
    for s in range(NS):
        bd[s * NK : (s + 1) * NK, s] = 1.0
    i2 = np.zeros((2, 4), np.float32)
    i2[0, 0] = i2[1, 1] = 1.0
    idmf = np.eye(C, dtype=np.float32)

    # I4S[dj][q, p*4+s] = 1 iff p == q-sh, p in [0,128), same 64-block
    i4 = np.zeros((C, KUP * NS * C), np.float32)
    for dj in range(KUP):
        sh = dj - 2
        for q in range(C):
            p = q - sh
            if 0 <= p < C and p // 64 == q // 64:
                for s in range(NS):
                    i4[q, dj * NS * C + p * NS + s] = 1.0
    i4 = i4.astype(NP_BF16)

    xp = np.pad(x, ((0, 0), (0, 0), (2, 2), (0, 0)))             # H-halo zeros
    in_maps = []
    for core in range(NCORES):
        b, half = core // 2, core % 2
        r0 = half * RH
        xs = np.zeros((C, 8 + XR * W), np.float32)
        xs[:, 4 : 4 + XR * W] = xp[b, :, r0 : r0 + XR, :].reshape(C, XR * W)
        # pixel-major bf16 transposes of the shard (halo included):
        # XTE[64*par+w, t*128+c] = xp[b, c, r0+2t+par, w]
        xsh = xp[b, :, r0 : r0 + XR, :].transpose(1, 2, 0)       # (36, 64, 128)
        xte = np.ascontiguousarray(
            xsh[0 : 2 * NTE].reshape(NTE, C, C).transpose(1, 0, 2)
            .reshape(C, NTE * C)).astype(NP_BF16)
        xto = np.ascontiguousarray(
            xsh[1 : 1 + 2 * NTO].reshape(NTO, C, C).transpose(1, 0, 2)
            .reshape(C, NTO * C)).astype(NP_BF16)
        be = np.zeros((CM, 4), np.float32)
        if half == 0:
            be[:, 0] = 0.0            # h row 0 = image row -1 -> zero
            be[:, 1] = 0.0
            be[:, 2] = inv
            be[:, 3] = bias
        else:
            be[:, 0] = inv
            be[:, 1] = bias
            be[:, 2] = 0.0            # h row HR-1 = image row 64 -> zero
            be[:, 3] = 0.0
        in_maps.append({
            "xs": xs.astype(NP_BF16), "xte": xte, "xto": xto, "i4": i4,
            "w1t": w1t, "w2l": w2l,
            "bns": inv.reshape(CM, 1).astype(np.float32),
            "bnb": bias.reshape(CM, 1).astype(np.float32),
            "be": be, "bd": bd,
            "i2": i2.astype(NP_BF16), "i2r": i2, "idmf": idmf,
        })
    return in_maps


def _assemble(results):
    out = np.zeros((B, C, 2 * H, 2 * W), np.float32)
    for core in range(NCORES):
        b, half = core // 2, core % 2
        o = results[core]["o"].reshape(C, 2 * RH, 2 * W)
        out[b, :, half * 2 * RH : (half + 1) * 2 * RH, :] = o
    return out


def kernel(x, w1, w2, bn_gamma, bn_beta, bn_mean, bn_var):
    nc = _build()
    in_maps = _host_prep(x, w1, w2, bn_gamma, bn_beta, bn_mean, bn_var)

    if os.environ.get("CARAFE_BACKEND", "hw") == "sim":
        from concourse.bass_interp import CoreSim
        results = []
        for core in range(NCORES):
            sim = CoreSim(nc)
            for name, arr in in_maps[core].items():
                sim.tensor(name)[:] = arr
            sim.simulate()
            results.append({"o": np.array(sim.mem_tensor("o"))})
    else:
        from concourse.bass_utils import run_bass_kernel_spmd
        res = run_bass_kernel_spmd(nc, in_maps, core_ids=list(range(NCORES)))
        results = res.results
    return _assemble(results)
